# revision 20
# baseline (speedup 1.0000x reference)
"""Trainium2 Bass kernel for a 2-layer GCN (GCNConv -> ReLU -> GCNConv).

Design (v3.3, ~1.7ms on 8 cores):
  - Nodes (dst) partitioned across 8 cores; tiny weights replicated.
    Aggregation commutes with the linear maps, so both layers aggregate
    64-wide bf16 features (dz = dinv*z for layer 1; o2 = t3 @ W2 for
    layer 2) and the W matmuls run per dst block.
  - Gather tables are the bf16 AllGather outputs directly, viewed as
    PAIR-rows (256B = 2 nodes); NQ=2 src halves keep the int16 gather
    index < 32768.  Per-edge dma_gather spread over 4 SWDGE queues.
  - Edges binned by (dst-chunk, src-half, dst-block), sorted by src
    parity; the even region is padded to 64 slots, the group to 128.
    Scatter-add runs on the tensor engine via one-hot S matrices
    (128-edge tile -> one full-row base-0 bf16 matmul into PSUM; the one
    parity-mixed tile per group uses two matmuls with S multiplied by
    static [128,1] partition masks — nonzero base partitions crash the
    runtime, so everything contracts from partition 0).
  - Self-loops never touch DMA: local dz / o2 block rows persist in SBUF
    and are added per block with one vector add.
  - Degrees/dinv and dz pre-scaling are computed on the host; per-core
    inputs ship compact (~2.5MB/core).
"""

import os
import sys
import types

import numpy as np
import ml_dtypes

try:
    import antenv.axon_hooks  # noqa: F401
except (ImportError, ModuleNotFoundError):
    try:
        import antenv
        _stub = types.ModuleType("antenv.axon_hooks")
        _stub.get_axon_ntff_profile_hook = lambda: None
        sys.modules["antenv.axon_hooks"] = _stub
        antenv.axon_hooks = _stub
    except ImportError:
        pass

import concourse.bass as bass
import concourse.mybir as mybir
import concourse.tile as tile
from concourse import bacc
from concourse import bass_utils

P = 128
NQ = 2  # src-half count (pair-row index limit: n_star/2/NQ < 32768)


class EdgePlan:
    def __init__(self, ncores, slice_sz, chunk_blocks):
        self.ncores = ncores
        self.slice_sz = slice_sz
        self.nblk = slice_sz // P
        self.n_star = ncores * slice_sz
        self.qsize = self.n_star // NQ          # nodes per half
        self.qrows = self.qsize // 2            # pair-rows per half
        self.chunk_blocks = chunk_blocks
        nch = -(-self.nblk // chunk_blocks)
        self.chunks = [list(range(c * chunk_blocks,
                                  min((c + 1) * chunk_blocks, self.nblk)))
                       for c in range(nch)]
        self.tcnt = None   # [nch, NQ, nblk]
        self.t0 = None
        self.keven = None  # [nch, NQ, nblk] #even-src edges in group
        self.TT = 0


def prep_edges(src, dst, ncores, slice_sz, chunk_blocks):
    """Bin edges by (dst-chunk, src-half, dst-block); pad groups to 128
    multiples (uniform across cores); sort each group by src parity.

    Returns (plan, g16, dstl8):
      g16:   [ncores, 16, TT*8] int16  pair-row index local to its half,
             dma_gather wrapped-16 layout
      dstl8: [ncores, P, TT] int8     dst offset within block, -1 = pad
    """
    pl = EdgePlan(ncores, slice_sz, chunk_blocks)
    nblk, nch, qsize = pl.nblk, len(pl.chunks), pl.qsize
    cb = chunk_blocks

    blk = dst // P
    core = blk // nblk
    b = blk % nblk
    ch = b // cb
    q = src // qsize
    parity = (src & 1).astype(np.int64)
    key = ((core * nch + ch) * NQ + q) * nblk + b
    order = np.lexsort((parity, key))
    key_s = key[order]
    src_s = src[order]
    dst_s = dst[order]
    par_s = parity[order]
    counts = np.bincount(key, minlength=ncores * nch * NQ * nblk)
    counts4 = counts.reshape(ncores, nch, NQ, nblk)
    tcnt = (-(-counts4 // P)).max(axis=0)  # [nch, NQ, nblk]

    # even-count per (core, group) -> per-tile parity split points; use the
    # per-core value (k differs per core, but the device code is identical
    # across cores, so splits must be uniform: use a COMMON split = max
    # even-count? No — ship per-core k as data is impossible (python-static).
    # Instead: pad each group so evens of all cores end at the same slot:
    # place evens at the front, odds at the BACK (descending from group
    # end), pads in the middle.  Then tile t's even-run length is
    # keven_core[t] and odd-run start is from the back — still per-core.
    # Simplest uniform scheme: split slots into an EVEN region and an ODD
    # region, each padded to the max over cores.
    even_cnt = np.bincount(key * 2 + (1 - parity),
                           minlength=2 * ncores * nch * NQ * nblk)
    even4 = even_cnt.reshape(ncores, nch, NQ, nblk, 2)[..., 1]
    odd4 = counts4 - even4
    # even region padded to 64 (matmul base partition must be 0/64/128-only
    # boundaries within a tile; 96 is forbidden by the PE quadrant-3 bug, so
    # 64-alignment is the finest legal grid).  Group padded to 128 overall.
    esz = (-(-even4.max(axis=0) // 64)) * 64
    gsz = (-(-(esz + odd4.max(axis=0)) // P)) * P
    tcnt = gsz // P
    pl.keven = esz  # even-slot count per group (uniform, 64-aligned)

    t0 = np.zeros_like(tcnt)
    t = 0
    for c in range(nch):
        for qq in range(NQ):
            for bb in pl.chunks[c]:
                t0[c, qq, bb] = t
                t += int(tcnt[c, qq, bb])
    pl.tcnt, pl.t0, pl.TT = tcnt, t0, t
    TT = pl.TT

    bstart = np.zeros(ncores * nch * NQ * nblk + 1, dtype=np.int64)
    bstart[1:] = np.cumsum(counts)
    ebstart = np.zeros(ncores * nch * NQ * nblk, dtype=np.int64)
    ebstart[:] = even4.reshape(-1)

    # slot position: evens at group offset rank; odds at keven + odd-rank
    rank = np.arange(len(key_s), dtype=np.int64) - bstart[key_s]
    erank = rank  # for evens (sorted parity-first => evens have rank < ecnt)
    orank = rank - ebstart[key_s]  # odd rank within group
    keven_flat = np.zeros(nch * NQ * nblk, dtype=np.int64)
    keven_flat[:] = pl.keven.reshape(-1)
    key_nc = key_s % (nch * NQ * nblk)
    slot_in_grp = np.where(par_s == 0, erank, keven_flat[key_nc] + orank)

    t0_flat = np.zeros(nch * NQ * nblk, dtype=np.int64)
    t0_flat[:] = t0.reshape(-1)
    pos = t0_flat[key_nc] * P + slot_in_grp
    core_s = key_s // (nch * NQ * nblk)

    gsrc = np.zeros((ncores, TT * P), dtype=np.int16)
    dstl = np.full((ncores, TT * P), -1, dtype=np.int8)
    rowloc = (src_s % qsize) // 2  # pair-row local to half
    gsrc[core_s, pos] = rowloc.astype(np.int16)
    dstl[core_s, pos] = (dst_s % P).astype(np.int8)

    dstl8 = np.ascontiguousarray(dstl.reshape(ncores, TT, P).transpose(0, 2, 1))
    g16 = np.ascontiguousarray(
        gsrc.reshape(ncores, TT * 8, 16).transpose(0, 2, 1))  # [nc, 16, TT*8]
    return pl, g16, dstl8


def build_kernel(pl, f_in, f_h, f_out, nqq=4, sp1=0):
    ncores, slice_sz, nblk = pl.ncores, pl.slice_sz, pl.nblk
    n_star, TT = pl.n_star, pl.TT
    qrows = pl.qrows
    dt = mybir.dt
    nch = len(pl.chunks)
    bf = dt.bfloat16

    def block_groups(c, bb):
        out = []
        for qq in range(NQ):
            n = int(pl.tcnt[c, qq, bb])
            if n:
                out.append((int(pl.t0[c, qq, bb]), n,
                            int(pl.keven[c, qq, bb])))
        return out

    max_grp = max(max((n for _, n, _ in block_groups(c, bb)), default=1)
                  for c in range(nch) for bb in pl.chunks[c])
    chunk_tiles = [sum(int(pl.tcnt[c, qq, bb]) for qq in range(NQ)
                       for bb in pl.chunks[c]) for c in range(nch)]
    max_ct = max(chunk_tiles)

    nc = bacc.Bacc("TRN2", target_bir_lowering=False, debug=False,
                   num_devices=ncores, num_swdge_queues=nqq)

    # --- I/O ---
    dzs_d = nc.dram_tensor("dzs", [slice_sz, f_in], bf, kind="ExternalInput")
    g16_d = nc.dram_tensor("g16", [16, TT * 8], dt.int16, kind="ExternalInput")
    dstl8_d = nc.dram_tensor("dstl8", [P, TT], dt.int8, kind="ExternalInput")
    dinvl_d = nc.dram_tensor("dinvl", [P, nblk], dt.float32,
                             kind="ExternalInput")
    W1_d = nc.dram_tensor("W1", [f_in, f_h], dt.float32, kind="ExternalInput")
    W2_d = nc.dram_tensor("W2", [f_h, f_out], dt.float32, kind="ExternalInput")
    b1r_d = nc.dram_tensor("b1r", [1, f_h], dt.float32, kind="ExternalInput")
    b2r_d = nc.dram_tensor("b2r", [1, f_out], dt.float32, kind="ExternalInput")
    y_d = nc.dram_tensor("y", [slice_sz, f_out], bf, kind="ExternalOutput")

    # --- internal DRAM ---
    # tables as pair-rows [n_star/2, 2*f] bf16 (256B rows for f=64)
    dzin_d = nc.dram_tensor("dzin", [slice_sz, f_in], bf)
    dzag_d = nc.dram_tensor("dzag", [n_star // 2, 2 * f_in], bf,
                            addr_space="Shared")
    o2p_d = nc.dram_tensor("o2p", [slice_sz, f_out], bf)
    o2ag_d = nc.dram_tensor("o2ag", [n_star // 2, 2 * f_out], bf,
                            addr_space="Shared")

    groups = [list(range(ncores))]

    def build_S(sp, gt0, gn):
        """One-hot S [P(edges), gn*P] bf16 for group tiles [gt0, gt0+gn)."""
        s_t = sp.tile([P, max_grp * P], bf, tag="sblk")
        out = s_t[:, :gn * P].rearrange("p (t j) -> p t j", t=gn)
        in0 = iota_t[:].unsqueeze(1).to_broadcast([P, gn, P])
        in1 = dstl_t[:, gt0:gt0 + gn].unsqueeze(2).to_broadcast([P, gn, P])
        nc.vector.tensor_tensor(out=out, in0=in0, in1=in1,
                                op=mybir.AluOpType.is_equal)
        return s_t

    def gather_chunk(gp, c, table_d, fw, qoff=0):
        """dma_gather chunk c from pair-row table (one call per half).
        fw = feature width per NODE; pair-row = 2*fw elems."""
        ct = chunk_tiles[c]
        ct0 = min(int(pl.t0[c, qq, bb]) for qq in range(NQ)
                  for bb in pl.chunks[c])
        gbuf = gp.tile([P, max_ct * 2 * fw], bf, tag="gbuf")
        for qq in range(NQ):
            qt = sum(int(pl.tcnt[c, qq, bb]) for bb in pl.chunks[c])
            if qt == 0:
                continue
            qt0 = min(int(pl.t0[c, qq, bb]) for bb in pl.chunks[c]
                      if pl.tcnt[c, qq, bb])
            n = qt * P
            nc.gpsimd.dma_gather(
                out_ap=gbuf[:, (qt0 - ct0) * 2 * fw:(qt0 - ct0 + qt) * 2 * fw]
                    .rearrange("p (t f) -> p t f", t=qt),
                in_ap=table_d[qq * qrows:(qq + 1) * qrows, :],
                idxs_ap=gidx_t[:, qt0 * 8:(qt0 + qt) * 8],
                num_idxs=n,
                num_idxs_reg=n,
                elem_size=2 * fw,
                single_packet=bool(sp1),
                queue_num=(c * NQ + qq + qoff) % nqq,
            )
        return gbuf, ct0

    def n_matmuls(grps):
        # one matmul per tile; the (single) parity-mixed tile needs two
        return sum(gn + (1 if kev % P else 0) for _, gn, kev in grps)

    def scatter_tiles(ho_ap, grps, ct0, gbuf, sp, fw, nmm):
        """Accumulate one dst block.  All matmuls contract over the full
        128 rows at base partition 0 (nonzero bases crash the runtime).
        Even-region rows read cols [0:fw), odd rows [fw:2fw); the one
        64-aligned mixed tile per group uses two parity-masked copies of
        its S column."""
        k = 0
        for gt0, gn, kev in grps:
            s_t = build_S(sp, gt0, gn)
            for t in range(gn):
                gcol = (gt0 - ct0 + t) * 2 * fw
                ke = min(max(kev - t * P, 0), P)
                scol = s_t[:, t * P:(t + 1) * P]
                if ke == P:      # pure even tile
                    ops = [(scol, 0)]
                elif ke == 0:    # pure odd tile
                    ops = [(scol, fw)]
                else:            # mixed at row 64: mask S by parity region
                    se = sp.tile([P, P], bf, tag="sxe")
                    nc.vector.tensor_tensor(
                        out=se[:], in0=scol,
                        in1=topm_t[:].to_broadcast([P, P]),
                        op=mybir.AluOpType.mult)
                    so = sp.tile([P, P], bf, tag="sxo")
                    nc.vector.tensor_tensor(
                        out=so[:], in0=scol,
                        in1=botm_t[:].to_broadcast([P, P]),
                        op=mybir.AluOpType.mult)
                    ops = [(se[:], 0), (so[:], fw)]
                for lhs, co in ops:
                    nc.tensor.matmul(
                        ho_ap, lhsT=lhs,
                        rhs=gbuf[:, gcol + co:gcol + co + fw],
                        start=(k == 0), stop=(k == nmm - 1))
                    k += 1

    with tile.TileContext(nc) as tc:
        with tc.tile_pool(name="persist", bufs=1) as pp:
            iota_t = pp.tile([P, P], dt.float32)
            ident_t = pp.tile([P, P], bf)
            ii32_t = pp.tile([P, P], dt.int32)
            pi32_t = pp.tile([P, 1], dt.int32)
            pio_t = pp.tile([P, 1], dt.float32)
            b1s_t = pp.tile([1, f_h], dt.float32)
            b2s_t = pp.tile([1, f_out], dt.float32)
            W1f_t = pp.tile([f_in, f_h], dt.float32)
            W2f_t = pp.tile([f_h, f_out], dt.float32)
            W1_t = pp.tile([f_in, f_h], bf)
            W2_t = pp.tile([f_h, f_out], bf)
            b1b_t = pp.tile([P, f_h], dt.float32)
            b2b_t = pp.tile([P, f_out], dt.float32)
            dinvl_t = pp.tile([P, nblk], dt.float32)
            dstl8_t = pp.tile([P, TT], dt.int8)
            dstl_t = pp.tile([P, TT], dt.float32)
            gidx_t = pp.tile([P, TT * 8], dt.int16)
            dzsb_t = pp.tile([P, nblk * f_in], bf)     # local dz slice
            o2keep_t = pp.tile([P, nblk * f_out], bf)  # local o2 slice
            topm_t = pp.tile([P, 1], bf)  # 1 on partitions < 64
            botm_t = pp.tile([P, 1], bf)  # 1 on partitions >= 64

            nc.gpsimd.iota(ii32_t[:], pattern=[[1, P]], base=0,
                           channel_multiplier=0)
            nc.vector.tensor_copy(iota_t[:], ii32_t[:])
            nc.gpsimd.iota(pi32_t[:], pattern=[[0, 1]], base=0,
                           channel_multiplier=1)
            nc.vector.tensor_copy(pio_t[:], pi32_t[:])
            nc.vector.tensor_tensor(out=ident_t[:], in0=iota_t[:],
                                    in1=pio_t[:].to_broadcast([P, P]),
                                    op=mybir.AluOpType.is_equal)
            nc.vector.tensor_scalar(out=topm_t[:], in0=pio_t[:],
                                    scalar1=float(P // 2), scalar2=None,
                                    op0=mybir.AluOpType.is_lt)
            nc.vector.tensor_scalar(out=botm_t[:], in0=pio_t[:],
                                    scalar1=float(P // 2 - 1), scalar2=None,
                                    op0=mybir.AluOpType.is_gt)
            nc.sync.dma_start(W1f_t[:], W1_d[:])
            nc.sync.dma_start(W2f_t[:], W2_d[:])
            nc.vector.tensor_copy(W1_t[:], W1f_t[:])
            nc.vector.tensor_copy(W2_t[:], W2f_t[:])
            nc.sync.dma_start(b1s_t[:], b1r_d[:])
            nc.sync.dma_start(b2s_t[:], b2r_d[:])
            nc.gpsimd.partition_broadcast(b1b_t[:], b1s_t[:])
            nc.gpsimd.partition_broadcast(b2b_t[:], b2s_t[:])
            nc.sync.dma_start(dinvl_t[:], dinvl_d[:])
            nc.sync.dma_start(dstl8_t[:], dstl8_d[:])
            nc.vector.tensor_copy(dstl_t[:], dstl8_t[:])
            for g in range(8):
                nc.sync.dma_start(gidx_t[g * 16:(g + 1) * 16, :], g16_d[:])
            nc.sync.dma_start(
                dzsb_t[:].rearrange("p (b f) -> p b f", b=nblk),
                dzs_d[:].rearrange("(b p) f -> p b f", p=P))

            nc.sync.dma_start(dzin_d[:], dzs_d[:])

            # ---------------- dz AllGather ----------------
            nc.gpsimd.collective_compute(
                "AllGather", mybir.AluOpType.bypass,
                replica_groups=groups,
                ins=[dzin_d[:].opt()],
                outs=[dzag_d[:].opt()])

            # ---------------- Layer 1 ----------------
            with tc.tile_pool(name="l1_g", bufs=6) as gp, \
                 tc.tile_pool(name="l1_s", bufs=6) as sp, \
                 tc.tile_pool(name="l1_e", bufs=6) as ep, \
                 tc.tile_pool(name="l1_u", bufs=6) as up, \
                 tc.tile_pool(name="l1_pst", bufs=3, space="PSUM") as pst, \
                 tc.tile_pool(name="l1_psh", bufs=5, space="PSUM") as psh:
                for c in range(nch):
                    gbuf, ct0 = gather_chunk(gp, c, dzag_d, f_in)
                    for bb in pl.chunks[c]:
                        grps = block_groups(c, bb)
                        ho = psh.tile([P, f_in + f_h + f_out], dt.float32,
                                      tag="aho")
                        aps1 = ho[:, :f_in]
                        scatter_tiles(aps1, grps, ct0, gbuf, sp, f_in,
                                      n_matmuls(grps))
                        # self-loop: agg += dz[block] (local, in SBUF)
                        u1p = up.tile([P, f_in], dt.float32, tag="u1p")
                        nc.vector.tensor_add(
                            u1p[:], aps1,
                            dzsb_t[:, bb * f_in:(bb + 1) * f_in])
                        # u1 = dinv_d * agg  [P, f_in] bf16
                        u1 = up.tile([P, f_in], bf, tag="u1")
                        nc.scalar.activation(
                            u1[:], u1p[:],
                            mybir.ActivationFunctionType.Copy,
                            scale=dinvl_t[:, bb:bb + 1])
                        psT = pst.tile([P, P], bf, tag="psT")
                        nc.tensor.transpose(psT[:f_in, :], u1[:], ident_t[:])
                        u1T = up.tile([f_in, P], bf, tag="u1T")
                        nc.vector.tensor_copy(u1T[:], psT[:f_in, :])
                        h1ps = ho[:, f_in:f_in + f_h]
                        nc.tensor.matmul(h1ps, lhsT=u1T[:], rhs=W1_t[:],
                                         start=True, stop=True)
                        t2 = ep.tile([P, f_h], dt.float32, tag="t2")
                        nc.vector.tensor_add(t2[:], h1ps, b1b_t[:])
                        t3 = ep.tile([P, f_h], bf, tag="t3")
                        nc.scalar.activation(
                            t3[:], t2[:],
                            mybir.ActivationFunctionType.Relu,
                            scale=dinvl_t[:, bb:bb + 1])
                        psT2 = pst.tile([P, P], bf, tag="psT")
                        nc.tensor.transpose(psT2[:], t3[:], ident_t[:])
                        u3T = up.tile([f_h, P], bf, tag="u3T")
                        nc.vector.tensor_copy(u3T[:], psT2[:])
                        o2ps = ho[:, f_in + f_h:]
                        nc.tensor.matmul(o2ps, lhsT=u3T[:], rhs=W2_t[:],
                                         start=True, stop=True)
                        o2sb = o2keep_t[:, bb * f_out:(bb + 1) * f_out]
                        nc.scalar.activation(
                            o2sb, o2ps,
                            mybir.ActivationFunctionType.Copy)
                        nc.sync.dma_start(
                            o2p_d[bb * P:(bb + 1) * P, :], o2sb)

            # ---------------- o2 AllGather ----------------
            nc.gpsimd.collective_compute(
                "AllGather", mybir.AluOpType.bypass,
                replica_groups=groups,
                ins=[o2p_d[:].opt()],
                outs=[o2ag_d[:].opt()])

            # ---------------- Layer 2 ----------------
            with tc.tile_pool(name="l2_g", bufs=6) as gp, \
                 tc.tile_pool(name="l2_s", bufs=6) as sp, \
                 tc.tile_pool(name="l2_e", bufs=6) as ep, \
                 tc.tile_pool(name="l2_ps", bufs=8, space="PSUM") as psa2:
                for c in range(nch):
                    gbuf, ct0 = gather_chunk(gp, c, o2ag_d, f_out, qoff=2)
                    for bb in pl.chunks[c]:
                        grps = block_groups(c, bb)
                        aps2 = psa2.tile([P, f_out], dt.float32, tag="aps2")
                        scatter_tiles(aps2[:], grps, ct0, gbuf, sp, f_out,
                                      n_matmuls(grps))
                        x1p = ep.tile([P, f_out], dt.float32, tag="x1p")
                        nc.vector.tensor_add(
                            x1p[:], aps2[:],
                            o2keep_t[:, bb * f_out:(bb + 1) * f_out])
                        x1 = ep.tile([P, f_out], dt.float32, tag="x1")
                        nc.scalar.activation(
                            x1[:], x1p[:],
                            mybir.ActivationFunctionType.Copy,
                            scale=dinvl_t[:, bb:bb + 1])
                        x2 = ep.tile([P, f_out], bf, tag="x2")
                        nc.vector.tensor_add(x2[:], x1[:], b2b_t[:])
                        nc.sync.dma_start(
                            y_d[bb * P:(bb + 1) * P, :], x2[:])

    nc.compile()
    return nc


def make_inputs(z, edge_index, W1, b1, W2, b2, ncores, slice_sz,
                chunk_blocks=4):
    n = z.shape[0]
    n_star = slice_sz * ncores
    f_in = z.shape[1]
    nblk = slice_sz // P

    src = np.asarray(edge_index[0], dtype=np.int64)
    dst = np.asarray(edge_index[1], dtype=np.int64)

    # self-loops are NOT binned as edges: the device adds the local dz/o2
    # block rows directly (they live in SBUF).  Degrees still include them.
    pl, g16, dstl8 = prep_edges(src, dst, ncores, slice_sz, chunk_blocks)

    loops = np.arange(n_star, dtype=np.int64)
    deg = np.bincount(np.concatenate([dst, loops]),
                      minlength=n_star).astype(np.float32)
    dinv = 1.0 / np.sqrt(np.maximum(deg, 1.0))

    zp = np.zeros((n_star, f_in), dtype=np.float32)
    zp[:n] = np.asarray(z, dtype=np.float32)
    dz = (dinv[:, None] * zp).astype(ml_dtypes.bfloat16)

    common = {
        "W1": np.ascontiguousarray(np.asarray(W1, dtype=np.float32)),
        "W2": np.ascontiguousarray(np.asarray(W2, dtype=np.float32)),
        "b1r": np.ascontiguousarray(np.asarray(b1, dtype=np.float32)[None, :]),
        "b2r": np.ascontiguousarray(np.asarray(b2, dtype=np.float32)[None, :]),
    }
    in_maps = []
    for c in range(ncores):
        m = dict(common)
        m["dzs"] = np.ascontiguousarray(dz[c * slice_sz:(c + 1) * slice_sz])
        m["g16"] = g16[c]
        m["dstl8"] = dstl8[c]
        m["dinvl"] = np.ascontiguousarray(
            dinv[c * slice_sz:(c + 1) * slice_sz].reshape(nblk, P).T)
        in_maps.append(m)
    return pl, in_maps


_CACHE = {}


def kernel(z, edge_index, W1, b1, W2, b2):
    NCORES = 8
    N = 100000
    SLICE = 12544

    pl, in_maps = make_inputs(z, edge_index, W1, b1, W2, b2, NCORES, SLICE)

    ck = (tuple(pl.tcnt.ravel().tolist()),
          tuple(pl.keven.ravel().tolist()), z.shape, edge_index.shape)
    if ck not in _CACHE:
        _CACHE[ck] = build_kernel(pl, f_in=z.shape[1], f_h=W1.shape[1],
                                  f_out=W2.shape[1])
    nc = _CACHE[ck]

    trace = bool(int(os.environ.get("KERNEL_TRACE", "0")))
    res = bass_utils.run_bass_kernel_spmd(
        nc, in_maps, core_ids=list(range(NCORES)), trace=trace)
    if res.exec_time_ns is not None:
        print(f"HW exec time: {res.exec_time_ns} ns")
        kernel.last_exec_time_ns = res.exec_time_ns
        kernel.last_trace = res.instructions_and_trace
    y = np.concatenate([np.asarray(r["y"], dtype=np.float32)
                        for r in res.results], axis=0)[:N]
    return np.ascontiguousarray(y, dtype=np.float32)
